# revision 5
# baseline (speedup 1.0000x reference)
"""FP4Linear on 8 TRN2 NeuronCores.

Computes out[B,S,Do] = x[B,S,Di] @ (codes[Do,Di] * s).T + bias[Do].

Sharding: tokens 4-way x out_features 2-way (each core gets a disjoint
[2048 tok, 2048 of] output block; x row-shards and W row-shards are
replicated across the matching axis). This halves per-core HBM reads vs
pure column-parallel (x would be fully replicated).

Per-core kernel (Tile framework):
  - W shard is shipped already transposed+packed on the host as fp8e4
    (int4 codes -8..7 are exactly representable in e4m3; the PE accepts
    an fp8 moving operand against the fp16 stationary x — verified
    bit-accurate on HW). DRAM layout [128 kpart, nof, kb_n*512] so
    resident SBUF tiles [128, kb_n, 512] fill via straight contiguous
    DMA — no xbar transposes for W at all. Chunk 0 is loaded in 8
    k-slices so the first matmuls unblock early.
  - x tile [128tok, 4096] is cast fp32->fp16 during the SWDGE DMA, then
    xbar-transposed SBUF->SBUF into [128k, 32kb, 128tok] on the sync
    HWDGE ring (which carries nothing else; both-rings xbar transposes
    race and corrupt). Tile 0 is loaded/transposed in 8 k-pieces, later
    tiles in halves, to keep the transpose pipeline ahead of the PE.
  - 32 fp16x fp8w matmuls accumulate per PSUM bank [128tok, 512of].
  - Eviction per 512-of chunk: ScalarE copy with per-partition scale AP
    (= weight_scale), VectorE bias add, store via the scalar HWDGE ring.
"""

import sys

import numpy as np

if "/opt/trn_rl_repo" not in sys.path:
    sys.path.insert(0, "/opt/trn_rl_repo")

import concourse.mybir as mybir  # noqa: E402
import concourse.tile as tile  # noqa: E402
from concourse import bacc  # noqa: E402
from concourse.bass_utils import run_bass_kernel_spmd  # noqa: E402

P = 128
MM_N = 512  # psum bank free dim (fp32)

N_CORES = 8
TOK_SHARDS = 4
OF_SHARDS = 2

# int4 code -> fp8e4 (e4m3) bit pattern, exact
_FP8_LUT = np.zeros(16, dtype=np.uint8)
for _c in range(-8, 8):
    import ml_dtypes as _mld

    _FP8_LUT[_c & 0xF] = np.float32(_c).astype(_mld.float8_e4m3).view(np.uint8)


def build_nc(tok: int, d_in: int, of: int):
    """One core's program: out[tok, of] = x[tok, d_in] @ w[of, d_in].T * s + b."""
    kb_n = d_in // P  # k blocks
    tt_n = tok // P  # token tiles
    nof = of // MM_N  # psum chunks along out features

    nc = bacc.Bacc("TRN2", target_bir_lowering=False)
    x_d = nc.dram_tensor("x", [tok, d_in], mybir.dt.float32, kind="ExternalInput")
    # pre-transposed on host: w[p, c, kb*512 + of_rel] = W[c*512+of_rel, kb*128+p]
    w_d = nc.dram_tensor(
        "w", [P, nof, kb_n * MM_N], mybir.dt.float8e4, kind="ExternalInput"
    )
    b_d = nc.dram_tensor("b", [of], mybir.dt.float32, kind="ExternalInput")
    s_d = nc.dram_tensor("s", [1], mybir.dt.float32, kind="ExternalInput")
    o_d = nc.dram_tensor("o", [tok, of], mybir.dt.float32, kind="ExternalOutput")

    with tile.TileContext(nc) as tc:
        with (
            tc.tile_pool(name="const", bufs=1) as cpool,
            tc.tile_pool(name="wt", bufs=1) as wtpool,
            tc.tile_pool(name="xin", bufs=5) as xpool,
            tc.tile_pool(name="xt", bufs=6) as xtpool,
            tc.tile_pool(name="out", bufs=8) as opool,
            tc.tile_pool(name="ps", bufs=8, space="PSUM") as pspool,
        ):
            wts = [
                wtpool.tile(
                    [P, kb_n, MM_N], mybir.dt.float8e4, tag=f"wt{c}", name=f"wt{c}"
                )
                for c in range(nof)
            ]

            def emit_x(t, splits=2):
                # SWDGE DMA casts fp32 -> fp16 in flight; sync-ring xbar
                # transpose into k-major. k-split loads/transposes unblock
                # the first matmuls of tile t after 1/splits of its bytes.
                x_nat = xpool.tile([P, d_in], mybir.dt.float16, tag="xnat")
                xt_t = xtpool.tile([P, kb_n, P], mybir.dt.float16, tag="xt")
                kq = d_in // splits
                kbq = kb_n // splits
                for q in range(splits):
                    nc.gpsimd.dma_start(
                        x_nat[:, q * kq : (q + 1) * kq],
                        x_d[t * P : (t + 1) * P, q * kq : (q + 1) * kq],
                    )
                    nc.sync.dma_start_transpose(
                        xt_t[:, q * kbq : (q + 1) * kbq, :],
                        x_nat[:, q * kq : (q + 1) * kq],
                    )
                return xt_t

            # x tile 0 in 8 pieces: first matmul gate is 1/8 of a tile.
            prefetched = {0: emit_x(0, splits=8)}

            # Constants ride the scalar HWDGE ring (fp32->fp32 broadcast
            # needs no cast): the Q7 SWDGE FIFO stays x-loads-only, so a
            # stalled x load can never delay the bias the evictions need.
            s_t = cpool.tile([P, 1], mybir.dt.float32, tag="s")
            nc.scalar.dma_start(s_t[:], s_d[None, :].to_broadcast((P, 1)))

            # W chunk 0 in 8 k-slices on the scalar HWDGE ring (deps are
            # AP-range granular, so MMs of kb block j wait only their slice).
            kb8 = kb_n // 8
            for piece in range(8):
                nc.scalar.dma_start(
                    wts[0][:, piece * kb8 : (piece + 1) * kb8, :],
                    w_d[:, 0, piece * kb8 * MM_N : (piece + 1) * kb8 * MM_N],
                )

            bias_t = cpool.tile([P, of], mybir.dt.float32, tag="bias")
            nc.scalar.dma_start(bias_t[:], b_d[None, :].to_broadcast((P, of)))

            for t in (1, 2, 3):
                prefetched[t] = emit_x(t)

            for c in range(1, nof):
                nc.scalar.dma_start(wts[c][:], w_d[:, c, :])

            for t in range(tt_n):
                xt_t = prefetched.pop(t) if t in prefetched else emit_x(t)

                for c in range(nof):
                    ps = pspool.tile([P, MM_N], mybir.dt.float32, tag="ps", name="ps")
                    for kb in range(kb_n):
                        nc.tensor.matmul(
                            ps[:],
                            xt_t[:, kb, :],
                            wts[c][:, kb, :],
                            start=(kb == 0),
                            stop=(kb == kb_n - 1),
                        )
                    # out = psum * s  (ACT copy, per-partition scale AP)
                    o_t = opool.tile([P, MM_N], mybir.dt.float32, tag="o", name="o_t")
                    nc.scalar.mul(o_t[:], ps[:], s_t[:, 0:1])
                    # out += bias (partition-broadcast), then store
                    nc.vector.tensor_add(
                        o_t[:], o_t[:], bias_t[:, c * MM_N : (c + 1) * MM_N]
                    )
                    nc.scalar.dma_start(
                        o_d[t * P : (t + 1) * P, c * MM_N : (c + 1) * MM_N], o_t[:]
                    )

    nc.compile()
    return nc


_NC_CACHE: dict = {}


def _get_nc(tok: int, d_in: int, of: int):
    key = (tok, d_in, of)
    if key not in _NC_CACHE:
        _NC_CACHE[key] = build_nc(tok, d_in, of)
    return _NC_CACHE[key]


def make_in_maps(x, fp4_weight, weight_scale, bias):
    """Shard full inputs into 8 per-core input maps."""
    b, s, d_in = x.shape
    d_out = fp4_weight.shape[0]
    tok = (b * s) // TOK_SHARDS
    of = d_out // OF_SHARDS
    nof = of // MM_N
    kb_n = d_in // P

    xf = np.ascontiguousarray(np.asarray(x, dtype=np.float32).reshape(b * s, d_in))
    # int4 codes -> exact fp8e4 bytes via LUT on the low nibble
    w8 = _FP8_LUT[np.asarray(fp4_weight, dtype=np.int32) & 0xF]
    b32 = np.ascontiguousarray(np.asarray(bias, dtype=np.float32))
    s32 = np.ascontiguousarray(np.asarray(weight_scale, dtype=np.float32).reshape(1))

    in_maps = []
    for core in range(N_CORES):
        ti, oi = divmod(core, OF_SHARDS)
        wsh = w8[oi * of : (oi + 1) * of]  # [of, d_in] uint8(e4m3 bits)
        # [c, of_rel, kb, p] -> [p, c, kb*512+of_rel]
        wt = wsh.reshape(nof, MM_N, kb_n, P).transpose(3, 0, 2, 1)
        wt = np.ascontiguousarray(wt.reshape(P, nof, kb_n * MM_N))
        in_maps.append(
            {
                "x": xf[ti * tok : (ti + 1) * tok],
                "w": wt,
                "b": b32[oi * of : (oi + 1) * of],
                "s": s32,
            }
        )
    return in_maps, (b, s, d_in, d_out, tok, of)


def kernel(x, fp4_weight, weight_scale, bias, **run_kwargs):
    in_maps, (b, s, d_in, d_out, tok, of) = make_in_maps(
        x, fp4_weight, weight_scale, bias
    )
    nc = _get_nc(tok, d_in, of)
    res = run_bass_kernel_spmd(nc, in_maps, core_ids=list(range(N_CORES)), **run_kwargs)

    out = np.empty((b * s, d_out), dtype=np.float32)
    for core in range(N_CORES):
        ti, oi = divmod(core, OF_SHARDS)
        out[ti * tok : (ti + 1) * tok, oi * of : (oi + 1) * of] = res.results[core]["o"]
    out = out.reshape(b, s, d_out)
    if run_kwargs:
        return out, res
    return out


# revision 6
# speedup vs baseline: 1.0362x; 1.0362x over previous
"""FP4Linear on 8 TRN2 NeuronCores.

Computes out[B,S,Do] = x[B,S,Di] @ (codes[Do,Di] * s).T + bias[Do].

Sharding: tokens 4-way x out_features 2-way (each core gets a disjoint
[2048 tok, 2048 of] output block; x row-shards and W row-shards are
replicated across the matching axis). This halves per-core HBM reads vs
pure column-parallel (x would be fully replicated).

Per-core kernel (Tile framework):
  - W shard is shipped already transposed+packed on the host as fp8e4
    (int4 codes -8..7 are exactly representable in e4m3; the PE accepts
    an fp8 moving operand against the fp16 stationary x — verified
    bit-accurate on HW). DRAM layout [128 kpart, nof, kb_n*512] so
    resident SBUF tiles [128, kb_n, 512] fill via straight contiguous
    DMA — no xbar transposes for W at all. Chunk 0 is loaded in 8
    k-slices so the first matmuls unblock early.
  - x tile [128tok, 4096] is cast fp32->fp16 during the SWDGE DMA, then
    xbar-transposed SBUF->SBUF into [128k, 32kb, 128tok] on the sync
    HWDGE ring (which carries nothing else; both-rings xbar transposes
    race and corrupt). Tile 0 is loaded/transposed in 8 k-pieces, later
    tiles in halves, to keep the transpose pipeline ahead of the PE.
  - Per PSUM bank [128tok, 512of]: a K=1 matmul of ones x (bias/s)
    seeds the accumulator with the (pre-scale) bias, then 32 fp16x fp8w
    matmuls accumulate. Folding the bias into PSUM removes both the
    [128, of] bias broadcast DMA (a recurring scheduling hazard) and
    the VectorE add.
  - Eviction per 512-of chunk: ScalarE copy with per-partition scale AP
    (= weight_scale) straight to the output tile, store via the scalar
    HWDGE ring.
"""

import sys

import numpy as np

if "/opt/trn_rl_repo" not in sys.path:
    sys.path.insert(0, "/opt/trn_rl_repo")

import ml_dtypes  # noqa: E402

import concourse.mybir as mybir  # noqa: E402
import concourse.tile as tile  # noqa: E402
from concourse import bacc  # noqa: E402
from concourse.bass_utils import run_bass_kernel_spmd  # noqa: E402

P = 128
MM_N = 512  # psum bank free dim (fp32)

N_CORES = 8
TOK_SHARDS = 4
OF_SHARDS = 2

# int4 code -> fp8e4 (e4m3) bit pattern, exact
_FP8_LUT = np.zeros(16, dtype=np.uint8)
for _c in range(-8, 8):
    _FP8_LUT[_c & 0xF] = np.float32(_c).astype(ml_dtypes.float8_e4m3).view(np.uint8)


def build_nc(tok: int, d_in: int, of: int):
    """One core's program: out[tok, of] = x[tok, d_in] @ w[of, d_in].T * s + b."""
    kb_n = d_in // P  # k blocks
    tt_n = tok // P  # token tiles
    nof = of // MM_N  # psum chunks along out features

    nc = bacc.Bacc("TRN2", target_bir_lowering=False)
    x_d = nc.dram_tensor("x", [tok, d_in], mybir.dt.float32, kind="ExternalInput")
    # pre-transposed on host: w[p, c, kb*512 + of_rel] = W[c*512+of_rel, kb*128+p]
    w_d = nc.dram_tensor(
        "w", [P, nof, kb_n * MM_N], mybir.dt.float8e4, kind="ExternalInput"
    )
    # bias/s as fp16 row (seeds PSUM via a K=1 ones-matmul)
    bs_d = nc.dram_tensor("bs", [1, of], mybir.dt.float16, kind="ExternalInput")
    one_d = nc.dram_tensor("one", [1, P], mybir.dt.float16, kind="ExternalInput")
    s_d = nc.dram_tensor("s", [1], mybir.dt.float32, kind="ExternalInput")
    o_d = nc.dram_tensor("o", [tok, of], mybir.dt.float32, kind="ExternalOutput")

    with tile.TileContext(nc) as tc:
        with (
            tc.tile_pool(name="const", bufs=1) as cpool,
            tc.tile_pool(name="wt", bufs=1) as wtpool,
            tc.tile_pool(name="xin", bufs=5) as xpool,
            tc.tile_pool(name="xt", bufs=6) as xtpool,
            tc.tile_pool(name="out", bufs=8) as opool,
            tc.tile_pool(name="ps", bufs=8, space="PSUM") as pspool,
        ):
            wts = [
                wtpool.tile(
                    [P, kb_n, MM_N], mybir.dt.float8e4, tag=f"wt{c}", name=f"wt{c}"
                )
                for c in range(nof)
            ]

            def emit_x(t, splits=2):
                # SWDGE DMA casts fp32 -> fp16 in flight; sync-ring xbar
                # transpose into k-major. k-split loads/transposes unblock
                # the first matmuls of tile t after 1/splits of its bytes.
                x_nat = xpool.tile([P, d_in], mybir.dt.float16, tag="xnat")
                xt_t = xtpool.tile([P, kb_n, P], mybir.dt.float16, tag="xt")
                kq = d_in // splits
                kbq = kb_n // splits
                for q in range(splits):
                    nc.gpsimd.dma_start(
                        x_nat[:, q * kq : (q + 1) * kq],
                        x_d[t * P : (t + 1) * P, q * kq : (q + 1) * kq],
                    )
                    nc.sync.dma_start_transpose(
                        xt_t[:, q * kbq : (q + 1) * kbq, :],
                        x_nat[:, q * kq : (q + 1) * kq],
                    )
                return xt_t

            # x tile 0 in 8 pieces: first matmul gate is 1/8 of a tile.
            prefetched = {0: emit_x(0, splits=8)}

            # Constants on the scalar HWDGE ring, ahead of everything else
            # there (tiny transfers; no Q7/stride-0 hazards).
            s_t = cpool.tile([P, 1], mybir.dt.float32, tag="s")
            nc.scalar.dma_start(s_t[:], s_d[None, :].to_broadcast((P, 1)))
            one_t = cpool.tile([1, P], mybir.dt.float16, tag="one")
            nc.scalar.dma_start(one_t[:], one_d[:])
            bs_t = cpool.tile([1, of], mybir.dt.float16, tag="bs")
            nc.scalar.dma_start(bs_t[:], bs_d[:])

            # W chunk 0 in 8 k-slices on the scalar HWDGE ring (deps are
            # AP-range granular, so MMs of kb block j wait only their slice).
            kb8 = kb_n // 8
            for piece in range(8):
                nc.scalar.dma_start(
                    wts[0][:, piece * kb8 : (piece + 1) * kb8, :],
                    w_d[:, 0, piece * kb8 * MM_N : (piece + 1) * kb8 * MM_N],
                )

            for t in (1, 2, 3):
                prefetched[t] = emit_x(t)

            for c in range(1, nof):
                nc.scalar.dma_start(wts[c][:], w_d[:, c, :])

            for t in range(tt_n):
                xt_t = prefetched.pop(t) if t in prefetched else emit_x(t)

                for c in range(nof):
                    ps = pspool.tile([P, MM_N], mybir.dt.float32, tag="ps", name="ps")
                    # seed psum with bias/s (broadcast along tokens via ones)
                    nc.tensor.matmul(
                        ps[:],
                        one_t[:],
                        bs_t[:, c * MM_N : (c + 1) * MM_N],
                        start=True,
                        stop=False,
                    )
                    for kb in range(kb_n):
                        nc.tensor.matmul(
                            ps[:],
                            xt_t[:, kb, :],
                            wts[c][:, kb, :],
                            start=False,
                            stop=(kb == kb_n - 1),
                        )
                    # out = psum * s  (ACT copy, per-partition scale AP)
                    o_t = opool.tile([P, MM_N], mybir.dt.float32, tag="o", name="o_t")
                    nc.scalar.mul(o_t[:], ps[:], s_t[:, 0:1])
                    nc.scalar.dma_start(
                        o_d[t * P : (t + 1) * P, c * MM_N : (c + 1) * MM_N], o_t[:]
                    )

    nc.compile()
    return nc


_NC_CACHE: dict = {}


def _get_nc(tok: int, d_in: int, of: int):
    key = (tok, d_in, of)
    if key not in _NC_CACHE:
        _NC_CACHE[key] = build_nc(tok, d_in, of)
    return _NC_CACHE[key]


def make_in_maps(x, fp4_weight, weight_scale, bias):
    """Shard full inputs into 8 per-core input maps."""
    b, s, d_in = x.shape
    d_out = fp4_weight.shape[0]
    tok = (b * s) // TOK_SHARDS
    of = d_out // OF_SHARDS
    nof = of // MM_N
    kb_n = d_in // P

    xf = np.ascontiguousarray(np.asarray(x, dtype=np.float32).reshape(b * s, d_in))
    # int4 codes -> exact fp8e4 bytes via LUT on the low nibble
    w8 = _FP8_LUT[np.asarray(fp4_weight, dtype=np.int32) & 0xF]
    s32 = np.ascontiguousarray(np.asarray(weight_scale, dtype=np.float32).reshape(1))
    sval = float(s32[0])
    bs16 = (np.asarray(bias, dtype=np.float64) / sval).astype(np.float16)
    ones = np.ones((1, P), dtype=np.float16)

    in_maps = []
    for core in range(N_CORES):
        ti, oi = divmod(core, OF_SHARDS)
        wsh = w8[oi * of : (oi + 1) * of]  # [of, d_in] uint8(e4m3 bits)
        # [c, of_rel, kb, p] -> [p, c, kb*512+of_rel]
        wt = wsh.reshape(nof, MM_N, kb_n, P).transpose(3, 0, 2, 1)
        wt = np.ascontiguousarray(wt.reshape(P, nof, kb_n * MM_N))
        in_maps.append(
            {
                "x": xf[ti * tok : (ti + 1) * tok],
                "w": wt,
                "bs": bs16[None, oi * of : (oi + 1) * of],
                "one": ones,
                "s": s32,
            }
        )
    return in_maps, (b, s, d_in, d_out, tok, of)


def kernel(x, fp4_weight, weight_scale, bias, **run_kwargs):
    in_maps, (b, s, d_in, d_out, tok, of) = make_in_maps(
        x, fp4_weight, weight_scale, bias
    )
    nc = _get_nc(tok, d_in, of)
    res = run_bass_kernel_spmd(nc, in_maps, core_ids=list(range(N_CORES)), **run_kwargs)

    out = np.empty((b * s, d_out), dtype=np.float32)
    for core in range(N_CORES):
        ti, oi = divmod(core, OF_SHARDS)
        out[ti * tok : (ti + 1) * tok, oi * of : (oi + 1) * of] = res.results[core]["o"]
    out = out.reshape(b, s, d_out)
    if run_kwargs:
        return out, res
    return out


# revision 7
# speedup vs baseline: 1.0917x; 1.0536x over previous
"""FP4Linear on 8 TRN2 NeuronCores.

Computes out[B,S,Do] = x[B,S,Di] @ (codes[Do,Di] * s).T + bias[Do].

Sharding: tokens 4-way x out_features 2-way (each core gets a disjoint
[2048 tok, 2048 of] output block; x row-shards and W row-shards are
replicated across the matching axis). This halves per-core HBM reads vs
pure column-parallel (x would be fully replicated).

Per-core kernel (Tile framework):
  - W shard is shipped already transposed+packed on the host as fp8e4
    (int4 codes -8..7 are exactly representable in e4m3; the PE accepts
    an fp8 moving operand against the fp16 stationary x — verified
    bit-accurate on HW). DRAM layout [128 kpart, nof, kb_n*512] so
    resident SBUF tiles [128, kb_n, 512] fill via straight contiguous
    DMA — no xbar transposes for W at all.
  - x tile [128tok, 4096] is cast fp32->fp16 during the SWDGE DMA, then
    xbar-transposed SBUF->SBUF into [128k, 32kb, 128tok] on the sync
    HWDGE ring (which carries nothing else; both-rings xbar transposes
    race and corrupt). Tile 0 goes in k-halves to shorten the startup
    gate.
  - DMA instruction count is kept low on purpose: completion tracking
    has only 8 DMAHW semaphore lanes shared by ALL queues, and a lane
    is only reusable once its previous DMA completed — too many small
    DMAs in flight serialize issue across unrelated queues (measured
    40us stalls from exactly this).
  - bias never touches a broadcast DMA: a one-time K=1 matmul of
    ones[1,128] x bias[1,512] per 512-chunk broadcasts it across
    partitions into PSUM, ScalarE copies it to a resident bias_t tile,
    and VectorE adds it per evicted chunk.
  - 32 fp16(x) x fp8(w) matmuls accumulate per PSUM bank [128t, 512of];
    eviction: ScalarE copy with per-partition scale AP (weight_scale)
    into a per-tile [128, 2048] out tile, one store per token tile via
    the scalar HWDGE ring.
"""

import sys

import numpy as np

if "/opt/trn_rl_repo" not in sys.path:
    sys.path.insert(0, "/opt/trn_rl_repo")

import ml_dtypes  # noqa: E402

import concourse.mybir as mybir  # noqa: E402
import concourse.tile as tile  # noqa: E402
from concourse import bacc  # noqa: E402
from concourse.bass_utils import run_bass_kernel_spmd  # noqa: E402

P = 128
MM_N = 512  # psum bank free dim (fp32)

N_CORES = 8
TOK_SHARDS = 4
OF_SHARDS = 2

# int4 code -> fp8e4 (e4m3) bit pattern, exact
_FP8_LUT = np.zeros(16, dtype=np.uint8)
for _c in range(-8, 8):
    _FP8_LUT[_c & 0xF] = np.float32(_c).astype(ml_dtypes.float8_e4m3).view(np.uint8)


def build_nc(tok: int, d_in: int, of: int):
    """One core's program: out[tok, of] = x[tok, d_in] @ w[of, d_in].T * s + b."""
    kb_n = d_in // P  # k blocks
    tt_n = tok // P  # token tiles
    nof = of // MM_N  # psum chunks along out features

    nc = bacc.Bacc("TRN2", target_bir_lowering=False)
    x_d = nc.dram_tensor("x", [tok, d_in], mybir.dt.float32, kind="ExternalInput")
    # pre-transposed on host: w[p, c, kb*512 + of_rel] = W[c*512+of_rel, kb*128+p]
    w_d = nc.dram_tensor(
        "w", [P, nof, kb_n * MM_N], mybir.dt.float8e4, kind="ExternalInput"
    )
    # packed constants row: [ones(P) | bias(of)] as fp16
    cst_d = nc.dram_tensor("cst", [1, P + of], mybir.dt.float16, kind="ExternalInput")
    s_d = nc.dram_tensor("s", [1], mybir.dt.float32, kind="ExternalInput")
    o_d = nc.dram_tensor("o", [tok, of], mybir.dt.float32, kind="ExternalOutput")

    with tile.TileContext(nc) as tc:
        with (
            tc.tile_pool(name="const", bufs=1) as cpool,
            tc.tile_pool(name="wt", bufs=1) as wtpool,
            tc.tile_pool(name="xin", bufs=5) as xpool,
            tc.tile_pool(name="xt", bufs=6) as xtpool,
            tc.tile_pool(name="out", bufs=2) as opool,
            tc.tile_pool(name="ps", bufs=8, space="PSUM") as pspool,
        ):
            wts = [
                wtpool.tile(
                    [P, kb_n, MM_N], mybir.dt.float8e4, tag=f"wt{c}", name=f"wt{c}"
                )
                for c in range(nof)
            ]

            def emit_x(t, splits=1):
                # SWDGE DMA casts fp32 -> fp16 in flight; sync-ring xbar
                # transpose into k-major.
                x_nat = xpool.tile([P, d_in], mybir.dt.float16, tag="xnat")
                xt_t = xtpool.tile([P, kb_n, P], mybir.dt.float16, tag="xt")
                kq = d_in // splits
                kbq = kb_n // splits
                for q in range(splits):
                    nc.gpsimd.dma_start(
                        x_nat[:, q * kq : (q + 1) * kq],
                        x_d[t * P : (t + 1) * P, q * kq : (q + 1) * kq],
                    )
                    nc.sync.dma_start_transpose(
                        xt_t[:, q * kbq : (q + 1) * kbq, :],
                        x_nat[:, q * kq : (q + 1) * kq],
                    )
                return xt_t

            # Constants in one small DMA at the head of the scalar ring.
            cst_t = cpool.tile([1, P + of], mybir.dt.float16, tag="cst")
            nc.scalar.dma_start(cst_t[:], cst_d[:])
            one_t = cst_t[:, 0:P]
            bias16 = cst_t[:, P : P + of]
            s_t = cpool.tile([P, 1], mybir.dt.float32, tag="s")
            nc.scalar.dma_start(s_t[:], s_d[None, :].to_broadcast((P, 1)))

            # x tile 0 in k-halves: first matmul gate is half a tile.
            prefetched = {0: emit_x(0, splits=2)}

            # W chunk 0 in halves, rest whole — few, large DMAs.
            h = kb_n // 2
            nc.scalar.dma_start(wts[0][:, :h, :], w_d[:, 0, : h * MM_N])
            nc.scalar.dma_start(wts[0][:, h:, :], w_d[:, 0, h * MM_N :])
            for c in range(1, nof):
                nc.scalar.dma_start(wts[c][:], w_d[:, c, :])

            for t in (1, 2, 3):
                prefetched[t] = emit_x(t)

            # One-time bias broadcast across partitions via K=1 matmuls,
            # parked in SBUF as fp32 [128, of]. No broadcast DMA involved.
            bias_t = cpool.tile([P, of], mybir.dt.float32, tag="bias")
            for c in range(nof):
                psb = pspool.tile([P, MM_N], mybir.dt.float32, tag="ps", name="ps")
                nc.tensor.matmul(
                    psb[:],
                    one_t,
                    bias16[:, c * MM_N : (c + 1) * MM_N],
                    start=True,
                    stop=True,
                )
                nc.scalar.copy(bias_t[:, c * MM_N : (c + 1) * MM_N], psb[:])

            for t in range(tt_n):
                xt_t = prefetched.pop(t) if t in prefetched else emit_x(t)

                o_t = opool.tile([P, of], mybir.dt.float32, tag="o", name="o_t")
                for c in range(nof):
                    ps = pspool.tile([P, MM_N], mybir.dt.float32, tag="ps", name="ps")
                    for kb in range(kb_n):
                        nc.tensor.matmul(
                            ps[:],
                            xt_t[:, kb, :],
                            wts[c][:, kb, :],
                            start=(kb == 0),
                            stop=(kb == kb_n - 1),
                        )
                    # out = psum * s  (ACT copy, per-partition scale AP)
                    nc.scalar.mul(
                        o_t[:, c * MM_N : (c + 1) * MM_N], ps[:], s_t[:, 0:1]
                    )
                    # out += bias (resident, broadcast once at startup)
                    nc.vector.tensor_add(
                        o_t[:, c * MM_N : (c + 1) * MM_N],
                        o_t[:, c * MM_N : (c + 1) * MM_N],
                        bias_t[:, c * MM_N : (c + 1) * MM_N],
                    )
                # one store per token tile
                nc.scalar.dma_start(o_d[t * P : (t + 1) * P, :], o_t[:])

    nc.compile()
    return nc


_NC_CACHE: dict = {}


def _get_nc(tok: int, d_in: int, of: int):
    key = (tok, d_in, of)
    if key not in _NC_CACHE:
        _NC_CACHE[key] = build_nc(tok, d_in, of)
    return _NC_CACHE[key]


def make_in_maps(x, fp4_weight, weight_scale, bias):
    """Shard full inputs into 8 per-core input maps."""
    b, s, d_in = x.shape
    d_out = fp4_weight.shape[0]
    tok = (b * s) // TOK_SHARDS
    of = d_out // OF_SHARDS
    nof = of // MM_N
    kb_n = d_in // P

    xf = np.ascontiguousarray(np.asarray(x, dtype=np.float32).reshape(b * s, d_in))
    # int4 codes -> exact fp8e4 bytes via LUT on the low nibble
    w8 = _FP8_LUT[np.asarray(fp4_weight, dtype=np.int32) & 0xF]
    s32 = np.ascontiguousarray(np.asarray(weight_scale, dtype=np.float32).reshape(1))
    b16 = np.asarray(bias, dtype=np.float32).astype(np.float16)

    in_maps = []
    for core in range(N_CORES):
        ti, oi = divmod(core, OF_SHARDS)
        wsh = w8[oi * of : (oi + 1) * of]  # [of, d_in] uint8(e4m3 bits)
        # [c, of_rel, kb, p] -> [p, c, kb*512+of_rel]
        wt = wsh.reshape(nof, MM_N, kb_n, P).transpose(3, 0, 2, 1)
        wt = np.ascontiguousarray(wt.reshape(P, nof, kb_n * MM_N))
        cst = np.concatenate(
            [np.ones(P, dtype=np.float16), b16[oi * of : (oi + 1) * of]]
        )[None, :]
        in_maps.append(
            {
                "x": xf[ti * tok : (ti + 1) * tok],
                "w": wt,
                "cst": np.ascontiguousarray(cst),
                "s": s32,
            }
        )
    return in_maps, (b, s, d_in, d_out, tok, of)


def kernel(x, fp4_weight, weight_scale, bias, **run_kwargs):
    in_maps, (b, s, d_in, d_out, tok, of) = make_in_maps(
        x, fp4_weight, weight_scale, bias
    )
    nc = _get_nc(tok, d_in, of)
    res = run_bass_kernel_spmd(nc, in_maps, core_ids=list(range(N_CORES)), **run_kwargs)

    out = np.empty((b * s, d_out), dtype=np.float32)
    for core in range(N_CORES):
        ti, oi = divmod(core, OF_SHARDS)
        out[ti * tok : (ti + 1) * tok, oi * of : (oi + 1) * of] = res.results[core]["o"]
    out = out.reshape(b, s, d_out)
    if run_kwargs:
        return out, res
    return out


# revision 8
# speedup vs baseline: 1.1919x; 1.0918x over previous
"""FP4Linear on 8 TRN2 NeuronCores.

Computes out[B,S,Do] = x[B,S,Di] @ (codes[Do,Di] * s).T + bias[Do].

Sharding: tokens 4-way x out_features 2-way (each core gets a disjoint
[2048 tok, 2048 of] output block; x row-shards and W row-shards are
replicated across the matching axis). This halves per-core HBM reads vs
pure column-parallel (x would be fully replicated).

Per-core kernel (Tile framework):
  - W shard is shipped already transposed+packed on the host as fp8e4
    (int4 codes -8..7 are exactly representable in e4m3; the PE accepts
    an fp8 moving operand against the fp16 stationary x — verified
    bit-accurate on HW). DRAM layout [128 kpart, nof, kb_n*512] so
    resident SBUF tiles [128, kb_n, 512] fill via straight contiguous
    DMA.
  - x is shipped host-packed in the k-major tile layout
    [128 kpart, tile, kb, 128 tok] (fp32, full precision preserved), so
    each token tile is ONE straight HWDGE DMA with 16KB/partition
    contiguous lines — no SWDGE cast DMAs and no SBUF xbar transposes
    at all (in earlier revisions those two flows double-handled every x
    byte through the shared SDMA pool and starved the PE during ramp).
    The otherwise-idle VectorE downcasts fp32 -> fp16 in SBUF, in
    k-halves so matmuls unblock after half a tile.
  - DMA instruction count is kept low on purpose: completion tracking
    has only 8 DMAHW semaphore lanes shared by ALL queues, and a lane
    is only reusable once its previous DMA completed — too many small
    DMAs in flight serialize issue across unrelated queues (measured
    40us stalls from exactly this).
  - bias never touches a broadcast DMA: a one-time K=1 matmul of
    ones[1,128] x bias[1,512] per 512-chunk broadcasts it across
    partitions into PSUM, ScalarE copies it to a resident bias_t tile,
    and VectorE adds it per evicted chunk.
  - 32 fp16(x) x fp8(w) matmuls accumulate per PSUM bank [128t, 512of];
    eviction: ScalarE copy with per-partition scale AP (weight_scale)
    into a per-tile [128, 2048] out tile, one store per token tile via
    the scalar HWDGE ring.
"""

import sys

import numpy as np

if "/opt/trn_rl_repo" not in sys.path:
    sys.path.insert(0, "/opt/trn_rl_repo")

import ml_dtypes  # noqa: E402

import concourse.mybir as mybir  # noqa: E402
import concourse.tile as tile  # noqa: E402
from concourse import bacc  # noqa: E402
from concourse.bass_utils import run_bass_kernel_spmd  # noqa: E402

P = 128
MM_N = 512  # psum bank free dim (fp32)

N_CORES = 8
TOK_SHARDS = 4
OF_SHARDS = 2

# int4 code -> fp8e4 (e4m3) bit pattern, exact
_FP8_LUT = np.zeros(16, dtype=np.uint8)
for _c in range(-8, 8):
    _FP8_LUT[_c & 0xF] = np.float32(_c).astype(ml_dtypes.float8_e4m3).view(np.uint8)


def build_nc(tok: int, d_in: int, of: int):
    """One core's program: out[tok, of] = x[tok, d_in] @ w[of, d_in].T * s + b."""
    kb_n = d_in // P  # k blocks
    tt_n = tok // P  # token tiles
    nof = of // MM_N  # psum chunks along out features

    nc = bacc.Bacc("TRN2", target_bir_lowering=False)
    # host-packed: x[p, t, kb, tok] = x_orig[t*128+tok, kb*128+p]
    x_d = nc.dram_tensor(
        "x", [P, tt_n, kb_n, P], mybir.dt.float32, kind="ExternalInput"
    )
    # pre-transposed on host: w[p, c, kb*512 + of_rel] = W[c*512+of_rel, kb*128+p]
    w_d = nc.dram_tensor(
        "w", [P, nof, kb_n * MM_N], mybir.dt.float8e4, kind="ExternalInput"
    )
    # packed constants row: [ones(P) | bias(of)] as fp16
    cst_d = nc.dram_tensor("cst", [1, P + of], mybir.dt.float16, kind="ExternalInput")
    s_d = nc.dram_tensor("s", [1], mybir.dt.float32, kind="ExternalInput")
    o_d = nc.dram_tensor("o", [tok, of], mybir.dt.float32, kind="ExternalOutput")

    with tile.TileContext(nc) as tc:
        with (
            tc.tile_pool(name="const", bufs=1) as cpool,
            tc.tile_pool(name="wt", bufs=1) as wtpool,
            tc.tile_pool(name="x32", bufs=3) as x32pool,
            tc.tile_pool(name="xt", bufs=5) as xtpool,
            tc.tile_pool(name="out", bufs=2) as opool,
            tc.tile_pool(name="ps", bufs=8, space="PSUM") as pspool,
        ):
            wts = [
                wtpool.tile(
                    [P, kb_n, MM_N], mybir.dt.float8e4, tag=f"wt{c}", name=f"wt{c}"
                )
                for c in range(nof)
            ]

            def emit_x(t, splits=1):
                # one straight HWDGE load (sync ring) + DVE downcast in halves
                x32 = x32pool.tile([P, kb_n, P], mybir.dt.float32, tag="x32")
                for q in range(splits):
                    kq = kb_n // splits
                    nc.sync.dma_start(
                        x32[:, q * kq : (q + 1) * kq, :],
                        x_d[:, t, q * kq : (q + 1) * kq, :],
                    )
                xt_t = xtpool.tile([P, kb_n, P], mybir.dt.float16, tag="xt")
                h = kb_n // 2
                nc.vector.tensor_copy(xt_t[:, :h, :], x32[:, :h, :])
                nc.vector.tensor_copy(xt_t[:, h:, :], x32[:, h:, :])
                return xt_t

            # Constants in one small DMA at the head of the scalar ring.
            cst_t = cpool.tile([1, P + of], mybir.dt.float16, tag="cst")
            nc.scalar.dma_start(cst_t[:], cst_d[:])
            one_t = cst_t[:, 0:P]
            bias16 = cst_t[:, P : P + of]
            s_t = cpool.tile([P, 1], mybir.dt.float32, tag="s")
            nc.scalar.dma_start(s_t[:], s_d[None, :].to_broadcast((P, 1)))

            # x tile 0 in k-halves: first matmul gate is half a tile.
            prefetched = {0: emit_x(0, splits=2)}

            # W chunk 0 in halves, rest whole — few, large DMAs.
            h = kb_n // 2
            nc.scalar.dma_start(wts[0][:, :h, :], w_d[:, 0, : h * MM_N])
            nc.scalar.dma_start(wts[0][:, h:, :], w_d[:, 0, h * MM_N :])
            for c in range(1, nof):
                nc.scalar.dma_start(wts[c][:], w_d[:, c, :])

            for t in (1, 2, 3):
                prefetched[t] = emit_x(t)

            # One-time bias broadcast across partitions via K=1 matmuls,
            # parked in SBUF as fp32 [128, of]. No broadcast DMA involved.
            bias_t = cpool.tile([P, of], mybir.dt.float32, tag="bias")
            for c in range(nof):
                psb = pspool.tile([P, MM_N], mybir.dt.float32, tag="ps", name="ps")
                nc.tensor.matmul(
                    psb[:],
                    one_t,
                    bias16[:, c * MM_N : (c + 1) * MM_N],
                    start=True,
                    stop=True,
                )
                nc.scalar.copy(bias_t[:, c * MM_N : (c + 1) * MM_N], psb[:])

            for t in range(tt_n):
                xt_t = prefetched.pop(t) if t in prefetched else emit_x(t)

                o_t = opool.tile([P, of], mybir.dt.float32, tag="o", name="o_t")
                for c in range(nof):
                    ps = pspool.tile([P, MM_N], mybir.dt.float32, tag="ps", name="ps")
                    for kb in range(kb_n):
                        nc.tensor.matmul(
                            ps[:],
                            xt_t[:, kb, :],
                            wts[c][:, kb, :],
                            start=(kb == 0),
                            stop=(kb == kb_n - 1),
                        )
                    # out = psum * s  (ACT copy, per-partition scale AP)
                    nc.scalar.mul(
                        o_t[:, c * MM_N : (c + 1) * MM_N], ps[:], s_t[:, 0:1]
                    )
                    # out += bias (resident, broadcast once at startup)
                    nc.vector.tensor_add(
                        o_t[:, c * MM_N : (c + 1) * MM_N],
                        o_t[:, c * MM_N : (c + 1) * MM_N],
                        bias_t[:, c * MM_N : (c + 1) * MM_N],
                    )
                # one store per token tile
                nc.scalar.dma_start(o_d[t * P : (t + 1) * P, :], o_t[:])

    nc.compile()
    return nc


_NC_CACHE: dict = {}


def _get_nc(tok: int, d_in: int, of: int):
    key = (tok, d_in, of)
    if key not in _NC_CACHE:
        _NC_CACHE[key] = build_nc(tok, d_in, of)
    return _NC_CACHE[key]


def make_in_maps(x, fp4_weight, weight_scale, bias):
    """Shard full inputs into 8 per-core input maps."""
    b, s, d_in = x.shape
    d_out = fp4_weight.shape[0]
    tok = (b * s) // TOK_SHARDS
    of = d_out // OF_SHARDS
    nof = of // MM_N
    kb_n = d_in // P
    tt_n = tok // P

    xf = np.asarray(x, dtype=np.float32).reshape(b * s, d_in)
    # int4 codes -> exact fp8e4 bytes via LUT on the low nibble
    w8 = _FP8_LUT[np.asarray(fp4_weight, dtype=np.int32) & 0xF]
    s32 = np.ascontiguousarray(np.asarray(weight_scale, dtype=np.float32).reshape(1))
    b16 = np.asarray(bias, dtype=np.float32).astype(np.float16)

    in_maps = []
    for core in range(N_CORES):
        ti, oi = divmod(core, OF_SHARDS)
        # x shard [tok, d_in] -> [p, t, kb, tok_rel]
        xs = xf[ti * tok : (ti + 1) * tok]
        xp = np.ascontiguousarray(
            xs.reshape(tt_n, P, kb_n, P).transpose(3, 0, 2, 1)
        )
        wsh = w8[oi * of : (oi + 1) * of]  # [of, d_in] uint8(e4m3 bits)
        # [c, of_rel, kb, p] -> [p, c, kb*512+of_rel]
        wt = wsh.reshape(nof, MM_N, kb_n, P).transpose(3, 0, 2, 1)
        wt = np.ascontiguousarray(wt.reshape(P, nof, kb_n * MM_N))
        cst = np.concatenate(
            [np.ones(P, dtype=np.float16), b16[oi * of : (oi + 1) * of]]
        )[None, :]
        in_maps.append(
            {
                "x": xp,
                "w": wt,
                "cst": np.ascontiguousarray(cst),
                "s": s32,
            }
        )
    return in_maps, (b, s, d_in, d_out, tok, of)


def kernel(x, fp4_weight, weight_scale, bias, **run_kwargs):
    in_maps, (b, s, d_in, d_out, tok, of) = make_in_maps(
        x, fp4_weight, weight_scale, bias
    )
    nc = _get_nc(tok, d_in, of)
    res = run_bass_kernel_spmd(nc, in_maps, core_ids=list(range(N_CORES)), **run_kwargs)

    out = np.empty((b * s, d_out), dtype=np.float32)
    for core in range(N_CORES):
        ti, oi = divmod(core, OF_SHARDS)
        out[ti * tok : (ti + 1) * tok, oi * of : (oi + 1) * of] = res.results[core]["o"]
    out = out.reshape(b, s, d_out)
    if run_kwargs:
        return out, res
    return out


# revision 10
# speedup vs baseline: 1.2315x; 1.0332x over previous
"""FP4Linear on 8 TRN2 NeuronCores.

Computes out[B,S,Do] = x[B,S,Di] @ (codes[Do,Di] * s).T + bias[Do].

Sharding: tokens 4-way x out_features 2-way (each core gets a disjoint
[2048 tok, 2048 of] output block; x row-shards and W row-shards are
replicated across the matching axis). This halves per-core HBM reads vs
pure column-parallel (x would be fully replicated).

Per-core kernel (Tile framework):
  - W shard is shipped already transposed+packed on the host as fp8e4
    (int4 codes -8..7 are exactly representable in e4m3; the PE accepts
    an fp8 moving operand against the fp16 stationary x — verified
    bit-accurate on HW). DRAM layout [128 kpart, nof, kb_n*512] so
    resident SBUF tiles [128, kb_n, 512] fill via straight contiguous
    DMA.
  - x is shipped host-packed in the k-major tile layout
    [128 kpart, tile, kb, 128 tok] (fp32, full precision preserved), so
    each token tile is ONE straight HWDGE DMA with 16KB/partition
    contiguous lines — no SWDGE cast DMAs and no SBUF xbar transposes
    at all (in earlier revisions those two flows double-handled every x
    byte through the shared SDMA pool and starved the PE during ramp).
    The otherwise-idle VectorE downcasts fp32 -> fp16 in SBUF, in
    k-halves so matmuls unblock after half a tile.
  - DMA instruction count is kept low on purpose: completion tracking
    has only 8 DMAHW semaphore lanes shared by ALL queues, and a lane
    is only reusable once its previous DMA completed — too many small
    DMAs in flight serialize issue across unrelated queues (measured
    40us stalls from exactly this).
  - bias never touches a broadcast DMA: a one-time K=1 matmul of
    ones[1,128] x bias[1,512] per 512-chunk broadcasts it across
    partitions into PSUM, ScalarE copies it to a resident bias_t tile,
    and VectorE adds it per evicted chunk.
  - 32 fp16(x) x fp8(w) matmuls accumulate per PSUM bank [128t, 512of];
    eviction: ScalarE copy with per-partition scale AP (weight_scale)
    into a per-tile [128, 2048] out tile, one store per token tile via
    the scalar HWDGE ring.
"""

import sys

import numpy as np

if "/opt/trn_rl_repo" not in sys.path:
    sys.path.insert(0, "/opt/trn_rl_repo")

import ml_dtypes  # noqa: E402

import concourse.mybir as mybir  # noqa: E402
import concourse.tile as tile  # noqa: E402
from concourse import bacc  # noqa: E402
from concourse.bass_utils import run_bass_kernel_spmd  # noqa: E402

P = 128
MM_N = 512  # psum bank free dim (fp32)

N_CORES = 8
TOK_SHARDS = 4
OF_SHARDS = 2

# int4 code -> fp8e4 (e4m3) bit pattern, exact
_FP8_LUT = np.zeros(16, dtype=np.uint8)
for _c in range(-8, 8):
    _FP8_LUT[_c & 0xF] = np.float32(_c).astype(ml_dtypes.float8_e4m3).view(np.uint8)


def build_nc(tok: int, d_in: int, of: int):
    """One core's program: out[tok, of] = x[tok, d_in] @ w[of, d_in].T * s + b."""
    kb_n = d_in // P  # k blocks
    tt_n = tok // P  # token tiles
    nof = of // MM_N  # psum chunks along out features

    nc = bacc.Bacc("TRN2", target_bir_lowering=False)
    # host-packed: x[p, t, kb, tok] = x_orig[t*128+tok, kb*128+p]
    x_d = nc.dram_tensor(
        "x", [P, tt_n, kb_n, P], mybir.dt.float32, kind="ExternalInput"
    )
    # pre-transposed on host: w[p, c, kb*512 + of_rel] = W[c*512+of_rel, kb*128+p]
    w_d = nc.dram_tensor(
        "w", [P, nof, kb_n * MM_N], mybir.dt.float8e4, kind="ExternalInput"
    )
    # packed constants row: [ones(P) | bias(of)] as fp16
    cst_d = nc.dram_tensor("cst", [1, P + of], mybir.dt.float16, kind="ExternalInput")
    s_d = nc.dram_tensor("s", [1], mybir.dt.float32, kind="ExternalInput")
    o_d = nc.dram_tensor("o", [tok, of], mybir.dt.float32, kind="ExternalOutput")

    with tile.TileContext(nc) as tc:
        with (
            tc.tile_pool(name="const", bufs=1) as cpool,
            tc.tile_pool(name="wt", bufs=1) as wtpool,
            tc.tile_pool(name="x32", bufs=3) as x32pool,
            tc.tile_pool(name="xt", bufs=5) as xtpool,
            tc.tile_pool(name="out", bufs=4) as opool,
            tc.tile_pool(name="ps", bufs=8, space="PSUM") as pspool,
        ):
            wts = [
                wtpool.tile(
                    [P, kb_n, MM_N], mybir.dt.float8e4, tag=f"wt{c}", name=f"wt{c}"
                )
                for c in range(nof)
            ]

            def emit_x(t, splits=1):
                # one straight HWDGE load (sync ring) + DVE downcast in halves
                x32 = x32pool.tile([P, kb_n, P], mybir.dt.float32, tag="x32")
                for q in range(splits):
                    kq = kb_n // splits
                    nc.sync.dma_start(
                        x32[:, q * kq : (q + 1) * kq, :],
                        x_d[:, t, q * kq : (q + 1) * kq, :],
                    )
                xt_t = xtpool.tile([P, kb_n, P], mybir.dt.float16, tag="xt")
                h = kb_n // 2
                nc.vector.tensor_copy(xt_t[:, :h, :], x32[:, :h, :])
                nc.vector.tensor_copy(xt_t[:, h:, :], x32[:, h:, :])
                return xt_t

            # Constants in one small DMA at the head of the scalar ring.
            cst_t = cpool.tile([1, P + of], mybir.dt.float16, tag="cst")
            nc.scalar.dma_start(cst_t[:], cst_d[:])
            one_t = cst_t[:, 0:P]
            bias16 = cst_t[:, P : P + of]
            s_t = cpool.tile([P, 1], mybir.dt.float32, tag="s")
            nc.scalar.dma_start(s_t[:], s_d[None, :].to_broadcast((P, 1)))

            # x tile 0 in k-halves: first matmul gate is half a tile.
            prefetched = {0: emit_x(0, splits=2)}

            # W chunk 0 in halves, rest whole — few, large DMAs.
            h = kb_n // 2
            nc.scalar.dma_start(wts[0][:, :h, :], w_d[:, 0, : h * MM_N])
            nc.scalar.dma_start(wts[0][:, h:, :], w_d[:, 0, h * MM_N :])
            for c in range(1, nof):
                nc.scalar.dma_start(wts[c][:], w_d[:, c, :])

            for t in (1, 2, 3):
                prefetched[t] = emit_x(t)

            # One-time bias broadcast across partitions via K=1 matmuls,
            # parked in SBUF as fp32 [128, of]. No broadcast DMA involved.
            bias_t = cpool.tile([P, of], mybir.dt.float32, tag="bias")
            for c in range(nof):
                psb = pspool.tile([P, MM_N], mybir.dt.float32, tag="ps", name="ps")
                nc.tensor.matmul(
                    psb[:],
                    one_t,
                    bias16[:, c * MM_N : (c + 1) * MM_N],
                    start=True,
                    stop=True,
                )
                nc.scalar.copy(bias_t[:, c * MM_N : (c + 1) * MM_N], psb[:])

            def chunk_mms(xt_t, o_t, t, c, store=False):
                ps = pspool.tile([P, MM_N], mybir.dt.float32, tag="ps", name="ps")
                for kb in range(kb_n):
                    nc.tensor.matmul(
                        ps[:],
                        xt_t[:, kb, :],
                        wts[c][:, kb, :],
                        start=(kb == 0),
                        stop=(kb == kb_n - 1),
                    )
                # out = psum * s  (ACT copy, per-partition scale AP)
                nc.scalar.mul(o_t[:, c * MM_N : (c + 1) * MM_N], ps[:], s_t[:, 0:1])
                # out += bias (resident, broadcast once at startup)
                nc.vector.tensor_add(
                    o_t[:, c * MM_N : (c + 1) * MM_N],
                    o_t[:, c * MM_N : (c + 1) * MM_N],
                    bias_t[:, c * MM_N : (c + 1) * MM_N],
                )
                if store:
                    nc.scalar.dma_start(
                        o_d[t * P : (t + 1) * P, c * MM_N : (c + 1) * MM_N],
                        o_t[:, c * MM_N : (c + 1) * MM_N],
                    )

            # Tiles 0-3 run chunk-major: W chunk c isn't needed until
            # ~4x later than tile-major order would demand, so the W DMAs
            # never gate the PE during ramp (x tiles are small and the
            # sync ring keeps 1 tile / ~8us pace easily).
            RAMP = min(4, tt_n)
            o_ramp = {
                t: opool.tile([P, of], mybir.dt.float32, tag="o", name="o_t")
                for t in range(RAMP)
            }
            for c in range(nof):
                for t in range(RAMP):
                    chunk_mms(prefetched[t], o_ramp[t], t, c)
            for t in range(RAMP):
                prefetched.pop(t)
                nc.scalar.dma_start(o_d[t * P : (t + 1) * P, :], o_ramp[t][:])

            for t in range(RAMP, tt_n):
                xt_t = prefetched.pop(t) if t in prefetched else emit_x(t)
                o_t = opool.tile([P, of], mybir.dt.float32, tag="o", name="o_t")
                last = t == tt_n - 1
                for c in range(nof):
                    # last tile: store per chunk to shorten the drain tail
                    chunk_mms(xt_t, o_t, t, c, store=last)
                if not last:
                    nc.scalar.dma_start(o_d[t * P : (t + 1) * P, :], o_t[:])

    nc.compile()
    return nc


_NC_CACHE: dict = {}


def _get_nc(tok: int, d_in: int, of: int):
    key = (tok, d_in, of)
    if key not in _NC_CACHE:
        _NC_CACHE[key] = build_nc(tok, d_in, of)
    return _NC_CACHE[key]


def make_in_maps(x, fp4_weight, weight_scale, bias):
    """Shard full inputs into 8 per-core input maps."""
    b, s, d_in = x.shape
    d_out = fp4_weight.shape[0]
    tok = (b * s) // TOK_SHARDS
    of = d_out // OF_SHARDS
    nof = of // MM_N
    kb_n = d_in // P
    tt_n = tok // P

    xf = np.asarray(x, dtype=np.float32).reshape(b * s, d_in)
    # int4 codes -> exact fp8e4 bytes via LUT on the low nibble
    w8 = _FP8_LUT[np.asarray(fp4_weight, dtype=np.int32) & 0xF]
    s32 = np.ascontiguousarray(np.asarray(weight_scale, dtype=np.float32).reshape(1))
    b16 = np.asarray(bias, dtype=np.float32).astype(np.float16)

    in_maps = []
    for core in range(N_CORES):
        ti, oi = divmod(core, OF_SHARDS)
        # x shard [tok, d_in] -> [p, t, kb, tok_rel]
        xs = xf[ti * tok : (ti + 1) * tok]
        xp = np.ascontiguousarray(
            xs.reshape(tt_n, P, kb_n, P).transpose(3, 0, 2, 1)
        )
        wsh = w8[oi * of : (oi + 1) * of]  # [of, d_in] uint8(e4m3 bits)
        # [c, of_rel, kb, p] -> [p, c, kb*512+of_rel]
        wt = wsh.reshape(nof, MM_N, kb_n, P).transpose(3, 0, 2, 1)
        wt = np.ascontiguousarray(wt.reshape(P, nof, kb_n * MM_N))
        cst = np.concatenate(
            [np.ones(P, dtype=np.float16), b16[oi * of : (oi + 1) * of]]
        )[None, :]
        in_maps.append(
            {
                "x": xp,
                "w": wt,
                "cst": np.ascontiguousarray(cst),
                "s": s32,
            }
        )
    return in_maps, (b, s, d_in, d_out, tok, of)


def kernel(x, fp4_weight, weight_scale, bias, **run_kwargs):
    in_maps, (b, s, d_in, d_out, tok, of) = make_in_maps(
        x, fp4_weight, weight_scale, bias
    )
    nc = _get_nc(tok, d_in, of)
    res = run_bass_kernel_spmd(nc, in_maps, core_ids=list(range(N_CORES)), **run_kwargs)

    out = np.empty((b * s, d_out), dtype=np.float32)
    for core in range(N_CORES):
        ti, oi = divmod(core, OF_SHARDS)
        out[ti * tok : (ti + 1) * tok, oi * of : (oi + 1) * of] = res.results[core]["o"]
    out = out.reshape(b, s, d_out)
    if run_kwargs:
        return out, res
    return out
